# Initial kernel scaffold
#
"""MoE layer (B=4,S=2048,D=1024,E=8,H=1024,top-2) on 8 trn2 NeuronCores.

Sharding: 4 token-groups x 2 expert-groups.
  core c: token group t = c % 4 (2048 tokens), expert group g = c // 4
  (experts 4g..4g+3). Host sums the two expert-group partials per token
  group and concatenates groups.

The expert axis is PERMUTED per core on the host (own experts first), so
the device code always treats experts 0..3 as local. The S-correction
column sums are mapped back to global order with an input permutation
matrix before the cross-core AllReduce.

Per-core pipeline (all on device):
  router (exact fp32: PE-transpose x tiles + matmuls) -> top-2 via max8 ->
  normalized weights via sigmoid(l1-l2) -> per-expert selection masks ->
  global S-correction sums via AllReduce (replicates the reference's
  torch-scatter artifact on tokens 0..7) -> prefix-sum slot assignment via
  triangular matmuls -> slot->token tables via one-hot matmuls -> FFN in
  fp32r with capacity 768/expert (512+256 slot groups) on gathered tokens
  -> combine by gathering each token's contribution rows back.
"""
import sys
import numpy as np
if "/opt/trn_rl_repo" not in sys.path:
    sys.path.insert(0, "/opt/trn_rl_repo")

B, S, D, E, H, TOPK = 4, 2048, 1024, 8, 1024, 2
N = B * S               # 8192 tokens
NC = 8                  # cores
TG = 4                  # token groups
NT = N // TG            # tokens per core = 2048
NTILE = NT // 128       # 16 token tiles
EPC = E // 2            # experts per core = 4
CAP = 768               # slot capacity per (core, expert); mean load 512
GROUPS = [(0, 512), (512, 256)]   # (start, size) slot groups per expert
NSLOT = EPC * CAP       # 3072 rows in compact buffer
CPE = CAP // 128        # slot chunks per expert = 6

_COMPILED = None
_GELU_OVERRIDE = None   # set to e.g. "Tanh" for CoreSim runs (no Gelu in sim)


def _build(reps=1):
    import contextlib
    import concourse.bass as bass
    import concourse.bacc as bacc
    import concourse.mybir as mybir
    from concourse.tile import TileContext
    from concourse.masks import make_identity

    f32 = mybir.dt.float32
    f32r = mybir.dt.float32r
    i32 = mybir.dt.int32
    u32 = mybir.dt.uint32
    AF = mybir.ActivationFunctionType
    ALU = mybir.AluOpType
    GELU = getattr(AF, _GELU_OVERRIDE) if _GELU_OVERRIDE else AF.Gelu

    nc = bacc.Bacc("TRN2", target_bir_lowering=False, debug=False, num_devices=NC)

    xg_d = nc.dram_tensor("xg", [NT, D], f32, kind="ExternalInput")
    wr_d = nc.dram_tensor("wr", [D, E], f32, kind="ExternalInput")
    rb_d = nc.dram_tensor("rb", [1, E], f32, kind="ExternalInput")
    w1_d = nc.dram_tensor("w1g", [EPC, D, H], f32r, kind="ExternalInput")
    b1_d = nc.dram_tensor("b1g", [EPC, H], f32, kind="ExternalInput")
    w2_d = nc.dram_tensor("w2g", [EPC, H, D], f32r, kind="ExternalInput")
    b2_d = nc.dram_tensor("b2g", [EPC, D], f32, kind="ExternalInput")
    ce_d = nc.dram_tensor("corr_en", [128, 1], f32, kind="ExternalInput")
    p8_d = nc.dram_tensor("p8", [E, E], f32, kind="ExternalInput")

    y_d = nc.dram_tensor("y", [NT, D], f32, kind="ExternalOutput")

    ycomp = nc.dram_tensor("ycomp", [NSLOT, D], f32)
    ar_in = nc.dram_tensor("ar_in", [1, 16], f32)
    ar_out = nc.dram_tensor("ar_out", [1, 16], f32, addr_space="Shared")

    xg_t = xg_d.rearrange("(f p) d -> f p d", p=128)
    y_t = y_d.rearrange("(f p) d -> f p d", p=128)

    with TileContext(nc) as tc, contextlib.ExitStack() as ctx:
        const = ctx.enter_context(tc.tile_pool(name="const", bufs=1))
        mpool = ctx.enter_context(tc.tile_pool(name="masks", bufs=1))
        w1pool = ctx.enter_context(tc.tile_pool(name="w1p", bufs=1))
        w2pool = ctx.enter_context(tc.tile_pool(name="w2p", bufs=1))
        big = ctx.enter_context(tc.tile_pool(name="big", bufs=4))
        sm = ctx.enter_context(tc.tile_pool(name="sm", bufs=3))
        ohp = ctx.enter_context(tc.tile_pool(name="ohp", bufs=2))
        gpool = ctx.enter_context(tc.tile_pool(name="gp", bufs=2))
        fpool = ctx.enter_context(tc.tile_pool(name="fp", bufs=1))

        # ---------------- constants ----------------
        ident = const.tile([128, 128], f32)
        make_identity(nc, ident[:])
        ones_c = const.tile([128, 1], f32)
        nc.vector.memset(ones_c[:], 1.0)
        ones_r = const.tile([1, 128], f32)
        nc.vector.memset(ones_r[:], 1.0)
        rowi = sm.tile([128, 128], i32, tag="it1")
        nc.gpsimd.iota(rowi[:], pattern=[[0, 128]], base=0, channel_multiplier=1)
        coli = sm.tile([128, 128], i32, tag="it2")
        nc.gpsimd.iota(coli[:], pattern=[[1, 128]], base=0, channel_multiplier=0)
        tril = const.tile([128, 128], f32)
        nc.vector.tensor_tensor(tril[:], rowi[:], coli[:], op=ALU.is_lt)
        it3 = sm.tile([128, CAP], i32, tag="it3")
        nc.gpsimd.iota(it3[:], pattern=[[1, CAP]], base=0, channel_multiplier=0)
        iota768 = const.tile([128, CAP], f32)
        nc.vector.tensor_copy(iota768[:], it3[:])
        it4 = sm.tile([128, 1], i32, tag="it4")
        nc.gpsimd.iota(it4[:], pattern=[[0, 1]], base=0, channel_multiplier=1)
        pidx = const.tile([128, 1], f32)
        nc.vector.tensor_copy(pidx[:], it4[:])
        it5 = sm.tile([128, NTILE], i32, tag="it5")
        nc.gpsimd.iota(it5[:], pattern=[[1, NTILE]], base=0, channel_multiplier=0)
        fvals = const.tile([128, NTILE], f32)
        nc.vector.tensor_copy(fvals[:], it5[:])
        ce = const.tile([128, 1], f32)
        nc.sync.dma_start(out=ce[:], in_=ce_d[:])
        p8sb = const.tile([E, E], f32)
        nc.sync.dma_start(out=p8sb[:], in_=p8_d[:])

        wrsb = const.tile([128, 8, E], f32)
        nc.sync.dma_start(out=wrsb[:], in_=wr_d.rearrange("(c p) e -> p c e", p=128))
        rbsb = const.tile([1, E], f32)
        nc.sync.dma_start(out=rbsb[:], in_=rb_d[:])
        b1sb = const.tile([128, EPC, 8], f32)
        nc.sync.dma_start(out=b1sb[:], in_=b1_d.rearrange("e (c p) -> p e c", p=128))
        b2sb = const.tile([1, EPC * D], f32)
        nc.sync.dma_start(out=b2sb[:], in_=b2_d.rearrange("e d -> (e d)")[None, :])

        for _rep in range(reps):
            # ---------------- router ----------------
            m1all = mpool.tile([128, NTILE * E], f32)
            m2all = mpool.tile([128, NTILE * E], f32)
            wr1 = mpool.tile([128, NTILE], f32)
            wr2 = mpool.tile([128, NTILE], f32)
            eid1 = mpool.tile([128, NTILE], f32)
            eid2 = mpool.tile([128, NTILE], f32)
            spart = mpool.tile([1, 16], f32)

            with (
                tc.tile_pool(name="ps_r1", bufs=3, space="PSUM") as ps_tp,
                tc.tile_pool(name="ps_r2", bufs=2, space="PSUM") as ps_lg,
                tc.tile_pool(name="ps_r3", bufs=1, space="PSUM") as ps_s,
            ):
                s1sb = sm.tile([1, E], f32, tag="s1sb")
                s2sb = sm.tile([1, E], f32, tag="s2sb")
                s1ps = ps_s.tile([1, E], f32, space="PSUM", tag="s1")
                s2ps = ps_s.tile([1, E], f32, space="PSUM", tag="s2")
                for f in range(NTILE):
                    xt = big.tile([128, 1024], f32, tag="bigbuf")
                    nc.sync.dma_start(out=xt[:], in_=xg_t[f])
                    xT = big.tile([128, 1024], f32, tag="bigbuf")
                    for c in range(8):
                        tp = ps_tp.tile([128, 128], f32, space="PSUM", tag="tp")
                        nc.tensor.transpose(out=tp[:], in_=xt[:, c * 128:(c + 1) * 128],
                                            identity=ident[:])
                        nc.vector.tensor_copy(xT[:, c * 128:(c + 1) * 128], tp[:])
                    lps = ps_lg.tile([128, E], f32, space="PSUM", tag="lps")
                    for c in range(8):
                        nc.tensor.matmul(lps[:], lhsT=xT[:, c * 128:(c + 1) * 128],
                                         rhs=wrsb[:, c, :],
                                         start=(c == 0), stop=False)
                    nc.tensor.matmul(lps[:], lhsT=ones_r[:], rhs=rbsb[:], start=False, stop=True)
                    lg = sm.tile([128, E], f32, tag="lg")
                    nc.vector.tensor_copy(lg[:], lps[:])

                    mx = sm.tile([128, 8], f32, tag="mx")
                    nc.vector.max(out=mx[:], in_=lg[:])
                    mix = sm.tile([128, 8], u32, tag="mix")
                    nc.vector.max_index(out=mix[:], in_max=mx[:], in_values=lg[:])
                    nc.vector.tensor_copy(eid1[:, f:f + 1], mix[:, 0:1])
                    nc.vector.tensor_copy(eid2[:, f:f + 1], mix[:, 1:2])

                    d12 = sm.tile([128, 1], f32, tag="d12")
                    nc.vector.tensor_sub(d12[:], mx[:, 0:1], mx[:, 1:2])
                    w1c = sm.tile([128, 1], f32, tag="w1c")
                    nc.scalar.activation(w1c[:], d12[:], AF.Sigmoid)
                    w2c = sm.tile([128, 1], f32, tag="w2c")
                    nc.vector.tensor_scalar(w2c[:], w1c[:], 1.0, scalar2=None, op0=ALU.subtract)
                    nc.vector.tensor_scalar(w2c[:], w2c[:], -1.0, scalar2=None, op0=ALU.mult)
                    nc.vector.tensor_copy(wr1[:, f:f + 1], w1c[:])
                    nc.vector.tensor_copy(wr2[:, f:f + 1], w2c[:])

                    eq1 = sm.tile([128, E], f32, tag="eq1")
                    nc.vector.tensor_tensor(eq1[:], lg[:], mx[:, 0:1].to_broadcast([128, E]),
                                            op=ALU.is_equal)
                    eq2 = sm.tile([128, E], f32, tag="eq2")
                    nc.vector.tensor_tensor(eq2[:], lg[:], mx[:, 1:2].to_broadcast([128, E]),
                                            op=ALU.is_equal)
                    m1f = m1all[:, f * E:(f + 1) * E]
                    m2f = m2all[:, f * E:(f + 1) * E]
                    nc.vector.tensor_tensor(m1f, eq1[:], w1c[:].to_broadcast([128, E]), op=ALU.mult)
                    nc.vector.tensor_tensor(m2f, eq2[:], w2c[:].to_broadcast([128, E]), op=ALU.mult)
                    nc.tensor.matmul(s1ps[:], lhsT=ones_c[:], rhs=m1f,
                                     start=(f == 0), stop=(f == NTILE - 1))
                    nc.tensor.matmul(s2ps[:], lhsT=ones_c[:], rhs=m2f,
                                     start=(f == 0), stop=(f == NTILE - 1))
                nc.vector.tensor_copy(s1sb[:], s1ps[:])
                nc.vector.tensor_copy(s2sb[:], s2ps[:])

            # local->global permute of S partials: s_global = s_localT.T @ P
            with tc.tile_pool(name="ps_sp", bufs=2, space="PSUM") as ps_sp:
                s1T_ps = ps_sp.tile([E, 1], f32, space="PSUM", tag="sT")
                nc.tensor.transpose(out=s1T_ps[:], in_=s1sb[:], identity=ident[0:1, 0:1])
                s1T = sm.tile([E, 1], f32, tag="s1T")
                nc.vector.tensor_copy(s1T[:], s1T_ps[:])
                s2T_ps = ps_sp.tile([E, 1], f32, space="PSUM", tag="sT")
                nc.tensor.transpose(out=s2T_ps[:], in_=s2sb[:], identity=ident[0:1, 0:1])
                s2T = sm.tile([E, 1], f32, tag="s2T")
                nc.vector.tensor_copy(s2T[:], s2T_ps[:])
                sg_ps = ps_sp.tile([1, E], f32, space="PSUM", tag="sg")
                nc.tensor.matmul(sg_ps[:], lhsT=s1T[:], rhs=p8sb[:], start=True, stop=True)
                nc.vector.tensor_copy(spart[:, 0:8], sg_ps[:])
                sg2_ps = ps_sp.tile([1, E], f32, space="PSUM", tag="sg")
                nc.tensor.matmul(sg2_ps[:], lhsT=s2T[:], rhs=p8sb[:], start=True, stop=True)
                nc.vector.tensor_copy(spart[:, 8:16], sg2_ps[:])

            # ---------------- S AllReduce + correction ----------------
            nc.sync.dma_start(out=ar_in[:], in_=spart[:])
            nc.gpsimd.collective_compute(
                "AllReduce", ALU.add, replica_groups=[list(range(NC))],
                ins=[ar_in[:]], outs=[ar_out[:]],
            )
            sglob = mpool.tile([1, 16], f32)
            nc.sync.dma_start(out=sglob[:], in_=ar_out[:])

            corrA = mpool.tile([8, 1], f32)
            corrB = mpool.tile([8, 1], f32)
            with tc.tile_pool(name="ps_c", bufs=2, space="PSUM") as ps_c:
                cA_ps = ps_c.tile([8, 1], f32, space="PSUM", tag="cA")
                nc.tensor.transpose(out=cA_ps[:], in_=sglob[:, 0:8], identity=ident[0:1, 0:1])
                nc.vector.tensor_tensor(corrA[:], cA_ps[:], ce[0:8, :], op=ALU.mult)
                cB_ps = ps_c.tile([8, 1], f32, space="PSUM", tag="cB")
                nc.tensor.transpose(out=cB_ps[:], in_=sglob[:, 8:16], identity=ident[0:1, 0:1])
                nc.vector.tensor_tensor(corrB[:], cB_ps[:], ce[0:8, :], op=ALU.mult)

            # ---------------- dispatch weights + slots per expert ----------------
            wd = []
            slots = []
            m1v = m1all[:].rearrange("p (f e) -> p e f", e=E)
            m2v = m2all[:].rearrange("p (f e) -> p e f", e=E)
            with (
                tc.tile_pool(name="ps_p1", bufs=2, space="PSUM") as ps_rp,
                tc.tile_pool(name="ps_p2", bufs=2, space="PSUM") as ps_cs,
            ):
                for le in range(EPC):
                    wde = mpool.tile([128, NTILE], f32, tag=f"wd{le}")
                    nc.vector.tensor_tensor(wde[:], m1v[:, le], m2v[:, le], op=ALU.add)
                    if le < 2:
                        corr = corrA if le == 0 else corrB
                        nc.vector.tensor_tensor(wde[0:8, 0:1], wde[0:8, 0:1], corr[:], op=ALU.add)
                    wd.append(wde)
                    sele = sm.tile([128, NTILE], f32, tag="sele")
                    nc.vector.tensor_scalar(sele[:], wde[:], 0.0, scalar2=None, op0=ALU.is_gt)

                    rp_ps = ps_rp.tile([128, NTILE], f32, space="PSUM", tag="rp")
                    nc.tensor.matmul(rp_ps[:], lhsT=tril[:], rhs=sele[:], start=True, stop=False)
                    cs_ps = ps_cs.tile([1, NTILE], f32, space="PSUM", tag="cs")
                    nc.tensor.matmul(cs_ps[:], lhsT=ones_c[:], rhs=sele[:], start=True, stop=True)
                    csum = sm.tile([1, NTILE], f32, tag="csum")
                    nc.vector.tensor_copy(csum[:], cs_ps[:])
                    for sh in (1, 2, 4, 8):
                        nc.vector.tensor_add(csum[:, sh:NTILE], csum[:, sh:NTILE],
                                             csum[:, 0:NTILE - sh])
                    excl = sm.tile([1, NTILE], f32, tag="excl")
                    nc.vector.memset(excl[:, 0:1], 0.0)
                    nc.vector.tensor_copy(excl[:, 1:NTILE], csum[:, 0:NTILE - 1])
                    nc.tensor.matmul(rp_ps[:], lhsT=ones_r[:], rhs=excl[:], start=False, stop=True)
                    sl = mpool.tile([128, NTILE], f32, tag=f"slot{le}")
                    nc.vector.tensor_copy(sl[:], rp_ps[:])
                    slots.append(sl)

            # ---------------- slot->token tables ----------------
            offs_id = []
            offs_w = []
            with (
                tc.tile_pool(name="ps_t1", bufs=2, space="PSUM") as ps_tb,
                tc.tile_pool(name="ps_t2", bufs=2, space="PSUM") as ps_t2,
            ):
                for le in range(EPC):
                    lha = sm.tile([128, NTILE * 3], f32r, tag="lha")
                    lhav = lha[:].rearrange("p (f three) -> p f three", three=3)
                    nc.vector.tensor_copy(lhav[:, :, 0], pidx[:].to_broadcast([128, NTILE]))
                    nc.vector.tensor_copy(lhav[:, :, 1], fvals[:])
                    nc.vector.tensor_copy(lhav[:, :, 2], wd[le][:])
                    oid = fpool.tile([128, CPE], i32, tag=f"oid{le}")
                    ow = fpool.tile([128, CPE], f32, tag=f"ow{le}")
                    for gi, (gstart, gsize) in enumerate(GROUPS):
                        tb_ps = ps_tb.tile([3, 512], f32, space="PSUM", tag="tb")
                        for f in range(NTILE):
                            oh = ohp.tile([128, 512], f32r, tag="oh")
                            nc.vector.tensor_tensor(
                                oh[:, 0:gsize],
                                slots[le][:, f:f + 1].to_broadcast([128, gsize]),
                                iota768[:, gstart:gstart + gsize], op=ALU.is_equal)
                            nc.tensor.matmul(tb_ps[:, 0:gsize], lhsT=lhav[:, f, :], rhs=oh[:, 0:gsize],
                                             start=(f == 0), stop=(f == NTILE - 1))
                        tbs = sm.tile([3, 512], f32, tag="tbs")
                        nc.vector.tensor_copy(tbs[:, 0:gsize], tb_ps[:, 0:gsize])
                        for ch in range(gsize // 128):
                            tp2 = ps_t2.tile([128, 3], f32, space="PSUM", tag="tp2")
                            nc.tensor.transpose(out=tp2[:], in_=tbs[:, ch * 128:(ch + 1) * 128],
                                                identity=ident[0:3, 0:3])
                            col = gstart // 128 + ch
                            # id = p + 128*f ; w
                            idf = sm.tile([128, 1], f32, tag="idf")
                            nc.vector.tensor_scalar(idf[:], tp2[:, 1:2], 128.0,
                                                    scalar2=None, op0=ALU.mult)
                            nc.vector.tensor_add(idf[:], idf[:], tp2[:, 0:1])
                            nc.vector.tensor_copy(oid[:, col:col + 1], idf[:])
                            nc.vector.tensor_copy(ow[:, col:col + 1], tp2[:, 2:3])
                    offs_id.append(oid)
                    offs_w.append(ow)

            # ---------------- FFN per expert ----------------
            with tc.tile_pool(name="ps_f", bufs=2, space="PSUM") as ps_f:
                for le in range(EPC):
                    w1sb = w1pool.tile([128, 8, H], f32r, tag="w1sb")
                    nc.sync.dma_start(out=w1sb[:], in_=w1_d[le].rearrange("(c p) h -> p c h", p=128))
                    w2sb = w2pool.tile([128, 8, D], f32r, tag="w2sb")
                    nc.scalar.dma_start(out=w2sb[:], in_=w2_d[le].rearrange("(c p) d -> p c d", p=128))

                    for gi, (gstart, gsize) in enumerate(GROUPS):
                        nch = gsize // 128
                        xinT = fpool.tile([128, 8 * 512], f32r, tag="ffa")
                        for sc in range(nch):
                            col = gstart // 128 + sc
                            xgt = gpool.tile([128, 1024], f32, tag="g")
                            nc.gpsimd.indirect_dma_start(
                                out=xgt[:], out_offset=None, in_=xg_d[:],
                                in_offset=bass.IndirectOffsetOnAxis(
                                    ap=offs_id[le][:, col:col + 1], axis=0))
                            xin = big.tile([128, 1024], f32, tag="bigbuf")
                            nc.scalar.activation(xin[:], xgt[:], AF.Copy,
                                                 scale=offs_w[le][:, col:col + 1])
                            for c in range(8):
                                tp = ps_f.tile([128, 128], f32, space="PSUM", tag="tpf")
                                nc.tensor.transpose(out=tp[:], in_=xin[:, c * 128:(c + 1) * 128],
                                                    identity=ident[:])
                                nc.vector.tensor_copy(
                                    xinT[:, c * gsize + sc * 128:c * gsize + (sc + 1) * 128], tp[:])
                        hT = fpool.tile([128, 8 * 512], f32r, tag="ffb")
                        for hc in range(8):
                            h_ps = ps_f.tile([128, 512], f32, space="PSUM", tag="h_ps")
                            for c in range(8):
                                nc.tensor.matmul(
                                    h_ps[:, 0:gsize],
                                    lhsT=w1sb[:, c, hc * 128:(hc + 1) * 128],
                                    rhs=xinT[:, c * gsize:(c + 1) * gsize],
                                    start=(c == 0), stop=(c == 7))
                            nc.scalar.activation(hT[:, hc * gsize:(hc + 1) * gsize],
                                                 h_ps[:, 0:gsize], GELU,
                                                 bias=b1sb[:, le, hc:hc + 1])
                        for sc in range(nch):
                            col = gstart // 128 + sc
                            yrow = big.tile([128, 1024], f32, tag="bigbuf")
                            for dh in range(2):
                                y_ps = ps_f.tile([128, 512], f32, space="PSUM", tag="y_ps")
                                for hc in range(8):
                                    nc.tensor.matmul(
                                        y_ps[:],
                                        lhsT=hT[:, hc * gsize + sc * 128:hc * gsize + (sc + 1) * 128],
                                        rhs=w2sb[:, hc, dh * 512:(dh + 1) * 512],
                                        start=(hc == 0), stop=False)
                                nc.tensor.matmul(
                                    y_ps[:], lhsT=ones_r[:],
                                    rhs=b2sb[:, le * D + dh * 512:le * D + (dh + 1) * 512],
                                    start=False, stop=True)
                                nc.vector.tensor_copy(yrow[:, dh * 512:(dh + 1) * 512], y_ps[:])
                            nc.sync.dma_start(
                                out=ycomp[(le * CPE + col) * 128:(le * CPE + col + 1) * 128, :],
                                in_=yrow[:])

            # ---------------- combine ----------------
            gs1 = mpool.tile([128, NTILE], f32, tag="gs1")
            gs2 = mpool.tile([128, NTILE], f32, tag="gs2")
            wm1 = mpool.tile([128, NTILE], f32, tag="wm1")
            wm2 = mpool.tile([128, NTILE], f32, tag="wm2")
            nc.vector.memset(gs1[:], float(NSLOT - 1))
            nc.vector.memset(gs2[:], float(NSLOT - 1))
            nc.vector.memset(wm1[:], 0.0)
            nc.vector.memset(wm2[:], 0.0)
            for le in range(EPC):
                for (gsx, wmx, eidx, wrx) in ((gs1, wm1, eid1, wr1), (gs2, wm2, eid2, wr2)):
                    eqt = sm.tile([128, NTILE], f32, tag="eqt")
                    nc.vector.tensor_scalar(eqt[:], eidx[:], float(le), scalar2=None,
                                            op0=ALU.is_equal)
                    tmp = sm.tile([128, NTILE], f32, tag="tmpa")
                    nc.vector.tensor_scalar(tmp[:], slots[le][:], float(le * CAP - (NSLOT - 1)),
                                            scalar2=None, op0=ALU.add)
                    nc.vector.tensor_tensor(tmp[:], tmp[:], eqt[:], op=ALU.mult)
                    nc.vector.tensor_add(gsx[:], gsx[:], tmp[:])
                    tmp2 = sm.tile([128, NTILE], f32, tag="tmpb")
                    nc.vector.tensor_tensor(tmp2[:], wrx[:], eqt[:], op=ALU.mult)
                    nc.vector.tensor_add(wmx[:], wmx[:], tmp2[:])
            gs1i = mpool.tile([128, NTILE], i32, tag="gs1i")
            nc.vector.tensor_copy(gs1i[:], gs1[:])
            gs2i = mpool.tile([128, NTILE], i32, tag="gs2i")
            nc.vector.tensor_copy(gs2i[:], gs2[:])

            for f in range(NTILE):
                acc = big.tile([128, 1024], f32, tag="bigbuf")
                if f == 0:
                    for le in range(EPC):
                        sl0 = sm.tile([128, 1], f32, tag="sl0")
                        nc.vector.tensor_scalar(sl0[:], slots[le][:, 0:1], float(le * CAP),
                                                scalar2=None, op0=ALU.add)
                        off0 = sm.tile([128, 1], i32, tag="off0")
                        nc.vector.tensor_copy(off0[:], sl0[:])
                        gt = gpool.tile([128, 1024], f32, tag="g")
                        nc.gpsimd.indirect_dma_start(
                            out=gt[:], out_offset=None, in_=ycomp[:],
                            in_offset=bass.IndirectOffsetOnAxis(ap=off0[:], axis=0))
                        scm = big.tile([128, 1024], f32, tag="bigbuf")
                        nc.scalar.activation(scm[:], gt[:], AF.Copy, scale=wd[le][:, 0:1])
                        if le == 0:
                            nc.vector.tensor_copy(acc[:], scm[:])
                        else:
                            nc.vector.tensor_add(acc[:], acc[:], scm[:])
                else:
                    g1 = gpool.tile([128, 1024], f32, tag="g")
                    nc.gpsimd.indirect_dma_start(
                        out=g1[:], out_offset=None, in_=ycomp[:],
                        in_offset=bass.IndirectOffsetOnAxis(ap=gs1i[:, f:f + 1], axis=0))
                    g2 = gpool.tile([128, 1024], f32, tag="g")
                    nc.gpsimd.indirect_dma_start(
                        out=g2[:], out_offset=None, in_=ycomp[:],
                        in_offset=bass.IndirectOffsetOnAxis(ap=gs2i[:, f:f + 1], axis=0))
                    nc.scalar.activation(acc[:], g1[:], AF.Copy, scale=wm1[:, f:f + 1])
                    s2t = big.tile([128, 1024], f32, tag="bigbuf")
                    nc.scalar.activation(s2t[:], g2[:], AF.Copy, scale=wm2[:, f:f + 1])
                    nc.vector.tensor_add(acc[:], acc[:], s2t[:])
                nc.sync.dma_start(out=y_t[f], in_=acc[:])

    nc.compile()
    return nc


def _get_compiled():
    global _COMPILED
    if _COMPILED is None:
        _COMPILED = _build()
    return _COMPILED


def _in_maps(inputs):
    x = np.asarray(inputs["inputs"], np.float32)
    wr = np.asarray(inputs["router_w"], np.float32)
    rb = np.asarray(inputs["router_b"], np.float32)
    w1 = np.asarray(inputs["w1"], np.float32)
    b1 = np.asarray(inputs["b1"], np.float32)
    w2 = np.asarray(inputs["w2"], np.float32)
    b2 = np.asarray(inputs["b2"], np.float32)
    flat = x.reshape(N, D)

    maps = []
    for c in range(NC):
        t = c % TG
        g = c // TG
        perm = list(range(g * EPC, g * EPC + EPC)) + \
               [e for e in range(E) if not (g * EPC <= e < g * EPC + EPC)]
        # p8 maps local S columns to global order; zeroed on the second
        # expert-group so the AllReduce counts every token exactly once.
        p8 = np.zeros((E, E), np.float32)
        if g == 0:
            for i_local, j_global in enumerate(perm):
                p8[i_local, j_global] = 1.0
        corr_en = np.zeros((128, 1), np.float32)
        if c == 0:
            corr_en[:E, 0] = 1.0
        maps.append({
            "xg": np.ascontiguousarray(flat[t * NT:(t + 1) * NT]),
            "wr": np.ascontiguousarray(wr[:, perm]),
            "rb": np.ascontiguousarray(rb[perm]).reshape(1, E),
            "w1g": np.ascontiguousarray(w1[g * EPC:(g + 1) * EPC]),
            "b1g": np.ascontiguousarray(b1[g * EPC:(g + 1) * EPC]),
            "w2g": np.ascontiguousarray(w2[g * EPC:(g + 1) * EPC]),
            "b2g": np.ascontiguousarray(b2[g * EPC:(g + 1) * EPC]),
            "corr_en": corr_en,
            "p8": p8,
        })
    return maps


def kernel(**inputs):
    nc = _get_compiled()
    maps = _in_maps(inputs)
    from concourse.bass_utils import run_bass_kernel_spmd
    res = run_bass_kernel_spmd(nc, maps, list(range(NC)))
    out = np.empty((N, D), np.float32)
    for t in range(TG):
        out[t * NT:(t + 1) * NT] = res.results[t]["y"] + res.results[t + TG]["y"]
    return out.reshape(B, S, D)



# revision 2
# speedup vs baseline: 1.0938x; 1.0938x over previous
"""MoE layer (B=4,S=2048,D=1024,E=8,H=1024,top-2) on 8 trn2 NeuronCores.

Sharding: 4 token-groups x 2 expert-groups.
  core c: token group t = c % 4 (2048 tokens), expert group g = c // 4
  (experts 4g..4g+3). Host sums the two expert-group partials per token
  group and concatenates groups.

The expert axis is PERMUTED per core on the host (own experts first), so
the device code always treats experts 0..3 as local. The S-correction
column sums are mapped back to global order with an input permutation
matrix before the cross-core AllReduce.

Per-core pipeline (all on device):
  router (exact fp32: PE-transpose x tiles + matmuls) -> top-2 via max8 ->
  normalized weights via sigmoid(l1-l2) -> per-expert selection masks ->
  global S-correction sums via AllReduce (replicates the reference's
  torch-scatter artifact on tokens 0..7) -> prefix-sum slot assignment via
  triangular matmuls -> slot->token tables via one-hot matmuls -> FFN in
  fp32r with capacity 768/expert (512+256 slot groups) on gathered tokens
  -> combine by gathering each token's contribution rows back.
"""
import sys
import numpy as np
if "/opt/trn_rl_repo" not in sys.path:
    sys.path.insert(0, "/opt/trn_rl_repo")

B, S, D, E, H, TOPK = 4, 2048, 1024, 8, 1024, 2
N = B * S               # 8192 tokens
NC = 8                  # cores
TG = 4                  # token groups
NT = N // TG            # tokens per core = 2048
NTILE = NT // 128       # 16 token tiles
EPC = E // 2            # experts per core = 4
CAP = 768               # slot capacity per (core, expert); mean load 512
GROUPS = [(0, 512), (512, 256)]   # (start, size) slot groups per expert
NSLOT = EPC * CAP       # 3072 rows in compact buffer
CPE = CAP // 128        # slot chunks per expert = 6

_COMPILED = None
_GELU_OVERRIDE = None   # set to e.g. "Tanh" for CoreSim runs (no Gelu in sim)


def _build(reps=1):
    import contextlib
    import concourse.bass as bass
    import concourse.bacc as bacc
    import concourse.mybir as mybir
    from concourse.tile import TileContext
    from concourse.masks import make_identity

    f32 = mybir.dt.float32
    f32r = mybir.dt.float32r
    i32 = mybir.dt.int32
    u32 = mybir.dt.uint32
    AF = mybir.ActivationFunctionType
    ALU = mybir.AluOpType
    GELU = getattr(AF, _GELU_OVERRIDE) if _GELU_OVERRIDE else AF.Gelu

    nc = bacc.Bacc("TRN2", target_bir_lowering=False, debug=False, num_devices=NC)

    xg_d = nc.dram_tensor("xg", [NT, D], f32, kind="ExternalInput")
    wr_d = nc.dram_tensor("wr", [D, E], f32, kind="ExternalInput")
    rb_d = nc.dram_tensor("rb", [1, E], f32, kind="ExternalInput")
    w1_d = nc.dram_tensor("w1g", [EPC, D, H], f32r, kind="ExternalInput")
    b1_d = nc.dram_tensor("b1g", [EPC, H], f32, kind="ExternalInput")
    w2_d = nc.dram_tensor("w2g", [EPC, H, D], f32r, kind="ExternalInput")
    b2_d = nc.dram_tensor("b2g", [EPC, D], f32, kind="ExternalInput")
    ce_d = nc.dram_tensor("corr_en", [128, 1], f32, kind="ExternalInput")
    p8_d = nc.dram_tensor("p8", [E, E], f32, kind="ExternalInput")

    y_d = nc.dram_tensor("y", [NT, D], f32, kind="ExternalOutput")

    ycomp = nc.dram_tensor("ycomp", [NSLOT, D], f32)
    ar_in = nc.dram_tensor("ar_in", [1, 16], f32)
    ar_out = nc.dram_tensor("ar_out", [1, 16], f32, addr_space="Shared")

    xg_t = xg_d.rearrange("(f p) d -> f p d", p=128)
    y_t = y_d.rearrange("(f p) d -> f p d", p=128)

    with TileContext(nc) as tc, contextlib.ExitStack() as ctx:
        const = ctx.enter_context(tc.tile_pool(name="const", bufs=1))
        mpool = ctx.enter_context(tc.tile_pool(name="masks", bufs=1))
        w1pool = ctx.enter_context(tc.tile_pool(name="w1p", bufs=1))
        w2pool = ctx.enter_context(tc.tile_pool(name="w2p", bufs=1))
        big = ctx.enter_context(tc.tile_pool(name="big", bufs=4))
        sm = ctx.enter_context(tc.tile_pool(name="sm", bufs=3))
        ohp = ctx.enter_context(tc.tile_pool(name="ohp", bufs=2))
        gpool = ctx.enter_context(tc.tile_pool(name="gp", bufs=2))
        fpool = ctx.enter_context(tc.tile_pool(name="fp", bufs=1))

        # ---------------- constants ----------------
        ident = const.tile([128, 128], f32)
        make_identity(nc, ident[:])
        ones_c = const.tile([128, 1], f32)
        nc.vector.memset(ones_c[:], 1.0)
        ones_r = const.tile([1, 128], f32)
        nc.vector.memset(ones_r[:], 1.0)
        rowi = sm.tile([128, 128], i32, tag="it1")
        nc.gpsimd.iota(rowi[:], pattern=[[0, 128]], base=0, channel_multiplier=1)
        coli = sm.tile([128, 128], i32, tag="it2")
        nc.gpsimd.iota(coli[:], pattern=[[1, 128]], base=0, channel_multiplier=0)
        tril = const.tile([128, 128], f32)
        nc.vector.tensor_tensor(tril[:], rowi[:], coli[:], op=ALU.is_lt)
        it3 = sm.tile([128, CAP], i32, tag="it3")
        nc.gpsimd.iota(it3[:], pattern=[[1, CAP]], base=0, channel_multiplier=0)
        iota768 = const.tile([128, CAP], f32)
        nc.vector.tensor_copy(iota768[:], it3[:])
        it4 = sm.tile([128, 1], i32, tag="it4")
        nc.gpsimd.iota(it4[:], pattern=[[0, 1]], base=0, channel_multiplier=1)
        pidx = const.tile([128, 1], f32)
        nc.vector.tensor_copy(pidx[:], it4[:])
        it5 = sm.tile([128, NTILE], i32, tag="it5")
        nc.gpsimd.iota(it5[:], pattern=[[1, NTILE]], base=0, channel_multiplier=0)
        fvals = const.tile([128, NTILE], f32)
        nc.vector.tensor_copy(fvals[:], it5[:])
        ce = const.tile([128, 1], f32)
        nc.sync.dma_start(out=ce[:], in_=ce_d[:])
        p8sb = const.tile([E, E], f32)
        nc.sync.dma_start(out=p8sb[:], in_=p8_d[:])

        wrsb = const.tile([128, 8, E], f32)
        nc.sync.dma_start(out=wrsb[:], in_=wr_d.rearrange("(c p) e -> p c e", p=128))
        rbsb = const.tile([1, E], f32)
        nc.sync.dma_start(out=rbsb[:], in_=rb_d[:])
        b1sb = const.tile([128, EPC, 8], f32)
        nc.sync.dma_start(out=b1sb[:], in_=b1_d.rearrange("e (c p) -> p e c", p=128))
        b2sb = const.tile([1, EPC * D], f32)
        nc.sync.dma_start(out=b2sb[:], in_=b2_d.rearrange("e d -> (e d)")[None, :])

        for _rep in range(reps):
            # ---------------- router ----------------
            rt_scope = nc.named_scope("router"); rt_scope.__enter__()
            m1all = mpool.tile([128, NTILE * E], f32)
            m2all = mpool.tile([128, NTILE * E], f32)
            wr1 = mpool.tile([128, NTILE], f32)
            wr2 = mpool.tile([128, NTILE], f32)
            eid1 = mpool.tile([128, NTILE], f32)
            eid2 = mpool.tile([128, NTILE], f32)
            spart = mpool.tile([1, 16], f32)

            with (
                tc.tile_pool(name="ps_r1", bufs=3, space="PSUM") as ps_tp,
                tc.tile_pool(name="ps_r2", bufs=2, space="PSUM") as ps_lg,
                tc.tile_pool(name="ps_r3", bufs=1, space="PSUM") as ps_s,
            ):
                s1sb = sm.tile([1, E], f32, tag="s1sb")
                s2sb = sm.tile([1, E], f32, tag="s2sb")
                s1ps = ps_s.tile([1, E], f32, space="PSUM", tag="s1")
                s2ps = ps_s.tile([1, E], f32, space="PSUM", tag="s2")
                for f in range(NTILE):
                    xt = big.tile([128, 1024], f32, tag="bigbuf")
                    nc.sync.dma_start(out=xt[:], in_=xg_t[f])
                    xT = big.tile([128, 1024], f32, tag="bigbuf")
                    for c in range(8):
                        tp = ps_tp.tile([128, 128], f32, space="PSUM", tag="tp")
                        nc.tensor.transpose(out=tp[:], in_=xt[:, c * 128:(c + 1) * 128],
                                            identity=ident[:])
                        nc.vector.tensor_copy(xT[:, c * 128:(c + 1) * 128], tp[:])
                    lps = ps_lg.tile([128, E], f32, space="PSUM", tag="lps")
                    for c in range(8):
                        nc.tensor.matmul(lps[:], lhsT=xT[:, c * 128:(c + 1) * 128],
                                         rhs=wrsb[:, c, :],
                                         start=(c == 0), stop=False)
                    nc.tensor.matmul(lps[:], lhsT=ones_r[:], rhs=rbsb[:], start=False, stop=True)
                    lg = sm.tile([128, E], f32, tag="lg")
                    nc.vector.tensor_copy(lg[:], lps[:])

                    mx = sm.tile([128, 8], f32, tag="mx")
                    nc.vector.max(out=mx[:], in_=lg[:])
                    mix = sm.tile([128, 8], u32, tag="mix")
                    nc.vector.max_index(out=mix[:], in_max=mx[:], in_values=lg[:])
                    nc.vector.tensor_copy(eid1[:, f:f + 1], mix[:, 0:1])
                    nc.vector.tensor_copy(eid2[:, f:f + 1], mix[:, 1:2])

                    d12 = sm.tile([128, 1], f32, tag="d12")
                    nc.vector.tensor_sub(d12[:], mx[:, 0:1], mx[:, 1:2])
                    w1c = sm.tile([128, 1], f32, tag="w1c")
                    nc.scalar.activation(w1c[:], d12[:], AF.Sigmoid)
                    w2c = sm.tile([128, 1], f32, tag="w2c")
                    nc.vector.tensor_scalar(w2c[:], w1c[:], 1.0, scalar2=None, op0=ALU.subtract)
                    nc.vector.tensor_scalar(w2c[:], w2c[:], -1.0, scalar2=None, op0=ALU.mult)
                    nc.vector.tensor_copy(wr1[:, f:f + 1], w1c[:])
                    nc.vector.tensor_copy(wr2[:, f:f + 1], w2c[:])

                    eq1 = sm.tile([128, E], f32, tag="eq1")
                    nc.vector.tensor_tensor(eq1[:], lg[:], mx[:, 0:1].to_broadcast([128, E]),
                                            op=ALU.is_equal)
                    eq2 = sm.tile([128, E], f32, tag="eq2")
                    nc.vector.tensor_tensor(eq2[:], lg[:], mx[:, 1:2].to_broadcast([128, E]),
                                            op=ALU.is_equal)
                    m1f = m1all[:, f * E:(f + 1) * E]
                    m2f = m2all[:, f * E:(f + 1) * E]
                    nc.vector.tensor_tensor(m1f, eq1[:], w1c[:].to_broadcast([128, E]), op=ALU.mult)
                    nc.vector.tensor_tensor(m2f, eq2[:], w2c[:].to_broadcast([128, E]), op=ALU.mult)
                    nc.tensor.matmul(s1ps[:], lhsT=ones_c[:], rhs=m1f,
                                     start=(f == 0), stop=(f == NTILE - 1))
                    nc.tensor.matmul(s2ps[:], lhsT=ones_c[:], rhs=m2f,
                                     start=(f == 0), stop=(f == NTILE - 1))
                nc.vector.tensor_copy(s1sb[:], s1ps[:])
                nc.vector.tensor_copy(s2sb[:], s2ps[:])

            # local->global permute of S partials: s_global = s_localT.T @ P
            with tc.tile_pool(name="ps_sp", bufs=2, space="PSUM") as ps_sp:
                s1T_ps = ps_sp.tile([E, 1], f32, space="PSUM", tag="sT")
                nc.tensor.transpose(out=s1T_ps[:], in_=s1sb[:], identity=ident[0:1, 0:1])
                s1T = sm.tile([E, 1], f32, tag="s1T")
                nc.vector.tensor_copy(s1T[:], s1T_ps[:])
                s2T_ps = ps_sp.tile([E, 1], f32, space="PSUM", tag="sT")
                nc.tensor.transpose(out=s2T_ps[:], in_=s2sb[:], identity=ident[0:1, 0:1])
                s2T = sm.tile([E, 1], f32, tag="s2T")
                nc.vector.tensor_copy(s2T[:], s2T_ps[:])
                sg_ps = ps_sp.tile([1, E], f32, space="PSUM", tag="sg")
                nc.tensor.matmul(sg_ps[:], lhsT=s1T[:], rhs=p8sb[:], start=True, stop=True)
                nc.vector.tensor_copy(spart[:, 0:8], sg_ps[:])
                sg2_ps = ps_sp.tile([1, E], f32, space="PSUM", tag="sg")
                nc.tensor.matmul(sg2_ps[:], lhsT=s2T[:], rhs=p8sb[:], start=True, stop=True)
                nc.vector.tensor_copy(spart[:, 8:16], sg2_ps[:])

            rt_scope.__exit__(None, None, None)
            ar_scope = nc.named_scope("allreduce"); ar_scope.__enter__()
            nc.sync.dma_start(out=ar_in[:], in_=spart[:])
            nc.gpsimd.collective_compute(
                "AllReduce", ALU.add, replica_groups=[list(range(NC))],
                ins=[ar_in[:]], outs=[ar_out[:]],
            )
            sglob = mpool.tile([1, 16], f32)
            nc.sync.dma_start(out=sglob[:], in_=ar_out[:])

            corrA = mpool.tile([8, 1], f32)
            corrB = mpool.tile([8, 1], f32)
            with tc.tile_pool(name="ps_c", bufs=2, space="PSUM") as ps_c:
                cA_ps = ps_c.tile([8, 1], f32, space="PSUM", tag="cA")
                nc.tensor.transpose(out=cA_ps[:], in_=sglob[:, 0:8], identity=ident[0:1, 0:1])
                nc.vector.tensor_tensor(corrA[:], cA_ps[:], ce[0:8, :], op=ALU.mult)
                cB_ps = ps_c.tile([8, 1], f32, space="PSUM", tag="cB")
                nc.tensor.transpose(out=cB_ps[:], in_=sglob[:, 8:16], identity=ident[0:1, 0:1])
                nc.vector.tensor_tensor(corrB[:], cB_ps[:], ce[0:8, :], op=ALU.mult)

            ar_scope.__exit__(None, None, None)
            sl_scope = nc.named_scope("slots"); sl_scope.__enter__()
            wd = []
            slots = []
            m1v = m1all[:].rearrange("p (f e) -> p e f", e=E)
            m2v = m2all[:].rearrange("p (f e) -> p e f", e=E)
            with (
                tc.tile_pool(name="ps_p1", bufs=2, space="PSUM") as ps_rp,
                tc.tile_pool(name="ps_p2", bufs=2, space="PSUM") as ps_cs,
            ):
                for le in range(EPC):
                    wde = mpool.tile([128, NTILE], f32, tag=f"wd{le}")
                    nc.vector.tensor_tensor(wde[:], m1v[:, le], m2v[:, le], op=ALU.add)
                    if le < 2:
                        corr = corrA if le == 0 else corrB
                        nc.vector.tensor_tensor(wde[0:8, 0:1], wde[0:8, 0:1], corr[:], op=ALU.add)
                    wd.append(wde)
                    sele = sm.tile([128, NTILE], f32, tag="sele")
                    nc.vector.tensor_scalar(sele[:], wde[:], 0.0, scalar2=None, op0=ALU.is_gt)

                    rp_ps = ps_rp.tile([128, NTILE], f32, space="PSUM", tag="rp")
                    nc.tensor.matmul(rp_ps[:], lhsT=tril[:], rhs=sele[:], start=True, stop=False)
                    cs_ps = ps_cs.tile([1, NTILE], f32, space="PSUM", tag="cs")
                    nc.tensor.matmul(cs_ps[:], lhsT=ones_c[:], rhs=sele[:], start=True, stop=True)
                    csum = sm.tile([1, NTILE], f32, tag="csum")
                    nc.vector.tensor_copy(csum[:], cs_ps[:])
                    for sh in (1, 2, 4, 8):
                        nc.vector.tensor_add(csum[:, sh:NTILE], csum[:, sh:NTILE],
                                             csum[:, 0:NTILE - sh])
                    excl = sm.tile([1, NTILE], f32, tag="excl")
                    nc.vector.memset(excl[:, 0:1], 0.0)
                    nc.vector.tensor_copy(excl[:, 1:NTILE], csum[:, 0:NTILE - 1])
                    nc.tensor.matmul(rp_ps[:], lhsT=ones_r[:], rhs=excl[:], start=False, stop=True)
                    sl = mpool.tile([128, NTILE], f32, tag=f"slot{le}")
                    nc.vector.tensor_copy(sl[:], rp_ps[:])
                    slots.append(sl)

            sl_scope.__exit__(None, None, None)
            tb_scope = nc.named_scope("tables"); tb_scope.__enter__()
            offs_id = []
            offs_w = []
            with (
                tc.tile_pool(name="ps_t1", bufs=2, space="PSUM") as ps_tb,
                tc.tile_pool(name="ps_t2", bufs=2, space="PSUM") as ps_t2,
            ):
                for le in range(EPC):
                    lha = sm.tile([128, NTILE * 3], f32r, tag="lha")
                    lhav = lha[:].rearrange("p (f three) -> p f three", three=3)
                    nc.vector.tensor_copy(lhav[:, :, 0], pidx[:].to_broadcast([128, NTILE]))
                    nc.vector.tensor_copy(lhav[:, :, 1], fvals[:])
                    nc.vector.tensor_copy(lhav[:, :, 2], wd[le][:])
                    oid = fpool.tile([128, CPE], i32, tag=f"oid{le}")
                    ow = fpool.tile([128, CPE], f32, tag=f"ow{le}")
                    for gi, (gstart, gsize) in enumerate(GROUPS):
                        tb_ps = ps_tb.tile([3, 512], f32, space="PSUM", tag="tb")
                        for f in range(NTILE):
                            oh = ohp.tile([128, 512], f32r, tag="oh")
                            nc.vector.tensor_tensor(
                                oh[:, 0:gsize],
                                slots[le][:, f:f + 1].to_broadcast([128, gsize]),
                                iota768[:, gstart:gstart + gsize], op=ALU.is_equal)
                            nc.tensor.matmul(tb_ps[:, 0:gsize], lhsT=lhav[:, f, :], rhs=oh[:, 0:gsize],
                                             start=(f == 0), stop=(f == NTILE - 1))
                        tbs = sm.tile([3, 512], f32, tag="tbs")
                        nc.vector.tensor_copy(tbs[:, 0:gsize], tb_ps[:, 0:gsize])
                        for ch in range(gsize // 128):
                            tp2 = ps_t2.tile([128, 3], f32, space="PSUM", tag="tp2")
                            nc.tensor.transpose(out=tp2[:], in_=tbs[:, ch * 128:(ch + 1) * 128],
                                                identity=ident[0:3, 0:3])
                            col = gstart // 128 + ch
                            # id = p + 128*f ; w
                            idf = sm.tile([128, 1], f32, tag="idf")
                            nc.vector.tensor_scalar(idf[:], tp2[:, 1:2], 128.0,
                                                    scalar2=None, op0=ALU.mult)
                            nc.vector.tensor_add(idf[:], idf[:], tp2[:, 0:1])
                            nc.vector.tensor_copy(oid[:, col:col + 1], idf[:])
                            nc.vector.tensor_copy(ow[:, col:col + 1], tp2[:, 2:3])
                    offs_id.append(oid)
                    offs_w.append(ow)

            tb_scope.__exit__(None, None, None)
            ff_scope = nc.named_scope("ffn"); ff_scope.__enter__()
            with tc.tile_pool(name="ps_f", bufs=2, space="PSUM") as ps_f:
                for le in range(EPC):
                    w1sb = w1pool.tile([128, 8, H], f32r, tag="w1sb")
                    nc.sync.dma_start(out=w1sb[:], in_=w1_d[le].rearrange("(c p) h -> p c h", p=128))
                    w2sb = w2pool.tile([128, 8, D], f32r, tag="w2sb")
                    nc.scalar.dma_start(out=w2sb[:], in_=w2_d[le].rearrange("(c p) d -> p c d", p=128))

                    for gi, (gstart, gsize) in enumerate(GROUPS):
                        nch = gsize // 128
                        xinT = fpool.tile([128, 8 * 512], f32r, tag="ffa")
                        for sc in range(nch):
                            col = gstart // 128 + sc
                            xgt = gpool.tile([128, 1024], f32, tag="g")
                            nc.gpsimd.indirect_dma_start(
                                out=xgt[:], out_offset=None, in_=xg_d[:],
                                in_offset=bass.IndirectOffsetOnAxis(
                                    ap=offs_id[le][:, col:col + 1], axis=0))
                            xin = big.tile([128, 1024], f32, tag="bigbuf")
                            nc.scalar.activation(xin[:], xgt[:], AF.Copy,
                                                 scale=offs_w[le][:, col:col + 1])
                            for c in range(8):
                                tp = ps_f.tile([128, 128], f32, space="PSUM", tag="tpf")
                                nc.tensor.transpose(out=tp[:], in_=xin[:, c * 128:(c + 1) * 128],
                                                    identity=ident[:])
                                nc.vector.tensor_copy(
                                    xinT[:, c * gsize + sc * 128:c * gsize + (sc + 1) * 128], tp[:])
                        hT = fpool.tile([128, 8 * 512], f32r, tag="ffb")
                        for hc in range(8):
                            h_ps = ps_f.tile([128, 512], f32, space="PSUM", tag="h_ps")
                            for c in range(8):
                                nc.tensor.matmul(
                                    h_ps[:, 0:gsize],
                                    lhsT=w1sb[:, c, hc * 128:(hc + 1) * 128],
                                    rhs=xinT[:, c * gsize:(c + 1) * gsize],
                                    start=(c == 0), stop=(c == 7))
                            nc.scalar.activation(hT[:, hc * gsize:(hc + 1) * gsize],
                                                 h_ps[:, 0:gsize], GELU,
                                                 bias=b1sb[:, le, hc:hc + 1])
                        for sc in range(nch):
                            col = gstart // 128 + sc
                            yrow = big.tile([128, 1024], f32, tag="bigbuf")
                            for dh in range(2):
                                y_ps = ps_f.tile([128, 512], f32, space="PSUM", tag="y_ps")
                                for hc in range(8):
                                    nc.tensor.matmul(
                                        y_ps[:],
                                        lhsT=hT[:, hc * gsize + sc * 128:hc * gsize + (sc + 1) * 128],
                                        rhs=w2sb[:, hc, dh * 512:(dh + 1) * 512],
                                        start=(hc == 0), stop=False)
                                nc.tensor.matmul(
                                    y_ps[:], lhsT=ones_r[:],
                                    rhs=b2sb[:, le * D + dh * 512:le * D + (dh + 1) * 512],
                                    start=False, stop=True)
                                nc.vector.tensor_copy(yrow[:, dh * 512:(dh + 1) * 512], y_ps[:])
                            nc.sync.dma_start(
                                out=ycomp[(le * CPE + col) * 128:(le * CPE + col + 1) * 128, :],
                                in_=yrow[:])

            ff_scope.__exit__(None, None, None)
            cb_scope = nc.named_scope("combine"); cb_scope.__enter__()
            gs1 = mpool.tile([128, NTILE], f32, tag="gs1")
            gs2 = mpool.tile([128, NTILE], f32, tag="gs2")
            wm1 = mpool.tile([128, NTILE], f32, tag="wm1")
            wm2 = mpool.tile([128, NTILE], f32, tag="wm2")
            nc.vector.memset(gs1[:], float(NSLOT - 1))
            nc.vector.memset(gs2[:], float(NSLOT - 1))
            nc.vector.memset(wm1[:], 0.0)
            nc.vector.memset(wm2[:], 0.0)
            for le in range(EPC):
                for (gsx, wmx, eidx, wrx) in ((gs1, wm1, eid1, wr1), (gs2, wm2, eid2, wr2)):
                    eqt = sm.tile([128, NTILE], f32, tag="eqt")
                    nc.vector.tensor_scalar(eqt[:], eidx[:], float(le), scalar2=None,
                                            op0=ALU.is_equal)
                    tmp = sm.tile([128, NTILE], f32, tag="tmpa")
                    nc.vector.tensor_scalar(tmp[:], slots[le][:], float(le * CAP - (NSLOT - 1)),
                                            scalar2=None, op0=ALU.add)
                    nc.vector.tensor_tensor(tmp[:], tmp[:], eqt[:], op=ALU.mult)
                    nc.vector.tensor_add(gsx[:], gsx[:], tmp[:])
                    tmp2 = sm.tile([128, NTILE], f32, tag="tmpb")
                    nc.vector.tensor_tensor(tmp2[:], wrx[:], eqt[:], op=ALU.mult)
                    nc.vector.tensor_add(wmx[:], wmx[:], tmp2[:])
            gs1i = mpool.tile([128, NTILE], i32, tag="gs1i")
            nc.vector.tensor_copy(gs1i[:], gs1[:])
            gs2i = mpool.tile([128, NTILE], i32, tag="gs2i")
            nc.vector.tensor_copy(gs2i[:], gs2[:])

            for f in range(NTILE):
                acc = big.tile([128, 1024], f32, tag="bigbuf")
                if f == 0:
                    for le in range(EPC):
                        sl0 = sm.tile([128, 1], f32, tag="sl0")
                        nc.vector.tensor_scalar(sl0[:], slots[le][:, 0:1], float(le * CAP),
                                                scalar2=None, op0=ALU.add)
                        off0 = sm.tile([128, 1], i32, tag="off0")
                        nc.vector.tensor_copy(off0[:], sl0[:])
                        gt = gpool.tile([128, 1024], f32, tag="g")
                        nc.gpsimd.indirect_dma_start(
                            out=gt[:], out_offset=None, in_=ycomp[:],
                            in_offset=bass.IndirectOffsetOnAxis(ap=off0[:], axis=0))
                        scm = big.tile([128, 1024], f32, tag="bigbuf")
                        nc.scalar.activation(scm[:], gt[:], AF.Copy, scale=wd[le][:, 0:1])
                        if le == 0:
                            nc.vector.tensor_copy(acc[:], scm[:])
                        else:
                            nc.vector.tensor_add(acc[:], acc[:], scm[:])
                else:
                    g1 = gpool.tile([128, 1024], f32, tag="g")
                    nc.gpsimd.indirect_dma_start(
                        out=g1[:], out_offset=None, in_=ycomp[:],
                        in_offset=bass.IndirectOffsetOnAxis(ap=gs1i[:, f:f + 1], axis=0))
                    g2 = gpool.tile([128, 1024], f32, tag="g")
                    nc.gpsimd.indirect_dma_start(
                        out=g2[:], out_offset=None, in_=ycomp[:],
                        in_offset=bass.IndirectOffsetOnAxis(ap=gs2i[:, f:f + 1], axis=0))
                    nc.scalar.activation(acc[:], g1[:], AF.Copy, scale=wm1[:, f:f + 1])
                    s2t = big.tile([128, 1024], f32, tag="bigbuf")
                    nc.scalar.activation(s2t[:], g2[:], AF.Copy, scale=wm2[:, f:f + 1])
                    nc.vector.tensor_add(acc[:], acc[:], s2t[:])
                nc.sync.dma_start(out=y_t[f], in_=acc[:])

            cb_scope.__exit__(None, None, None)
    nc.compile()
    return nc


def _get_compiled():
    global _COMPILED
    if _COMPILED is None:
        _COMPILED = _build()
    return _COMPILED


def _in_maps(inputs):
    x = np.asarray(inputs["inputs"], np.float32)
    wr = np.asarray(inputs["router_w"], np.float32)
    rb = np.asarray(inputs["router_b"], np.float32)
    w1 = np.asarray(inputs["w1"], np.float32)
    b1 = np.asarray(inputs["b1"], np.float32)
    w2 = np.asarray(inputs["w2"], np.float32)
    b2 = np.asarray(inputs["b2"], np.float32)
    flat = x.reshape(N, D)

    maps = []
    for c in range(NC):
        t = c % TG
        g = c // TG
        perm = list(range(g * EPC, g * EPC + EPC)) + \
               [e for e in range(E) if not (g * EPC <= e < g * EPC + EPC)]
        # p8 maps local S columns to global order; zeroed on the second
        # expert-group so the AllReduce counts every token exactly once.
        p8 = np.zeros((E, E), np.float32)
        if g == 0:
            for i_local, j_global in enumerate(perm):
                p8[i_local, j_global] = 1.0
        corr_en = np.zeros((128, 1), np.float32)
        if c == 0:
            corr_en[:E, 0] = 1.0
        maps.append({
            "xg": np.ascontiguousarray(flat[t * NT:(t + 1) * NT]),
            "wr": np.ascontiguousarray(wr[:, perm]),
            "rb": np.ascontiguousarray(rb[perm]).reshape(1, E),
            "w1g": np.ascontiguousarray(w1[g * EPC:(g + 1) * EPC]),
            "b1g": np.ascontiguousarray(b1[g * EPC:(g + 1) * EPC]),
            "w2g": np.ascontiguousarray(w2[g * EPC:(g + 1) * EPC]),
            "b2g": np.ascontiguousarray(b2[g * EPC:(g + 1) * EPC]),
            "corr_en": corr_en,
            "p8": p8,
        })
    return maps


def kernel(**inputs):
    nc = _get_compiled()
    maps = _in_maps(inputs)
    from concourse.bass_utils import run_bass_kernel_spmd
    res = run_bass_kernel_spmd(nc, maps, list(range(NC)))
    out = np.empty((N, D), np.float32)
    for t in range(TG):
        out[t * NT:(t + 1) * NT] = res.results[t]["y"] + res.results[t + TG]["y"]
    return out.reshape(B, S, D)

